# revision 14
# baseline (speedup 1.0000x reference)
"""DCNv3 Trainium2 Bass kernel v2 — data-parallel over batch (1 image per core).

Structure (vs v1): channels-on-partitions [C=2x128, t=H*W]; spatial shifts are
free-dim AP offsets into zero-padded flat buffers.
  - in_proj / depthwise conv (diagonal matmuls) / LN (ones-matmul channel sums)
    / GELU on PE+ACT+DVE.  Input x uploaded bf16, contiguous DMA, on-chip
    scatter into the padded layout.
  - bilinear sampling as 5x5 dynamic local window: 25 weight planes W[plane,g,t]
    built by a selection matmul (scand) over packed tent products.
  - W group->channel broadcast via SBUF->SBUF DMA with a stride-0 source dim
    (src [8part,(0,16),(1,1536)] -> dst [128part,(1,1536)]), replacing v1's
    400 PE broadcast matmuls + ACT PSUM drains.
  - plane slot order: slots 3u+ci (ci: v=-2,0,2; odd buffer) for u=0..4, then
    15+2u+e (v=-1,+1; even buffer), so the 10 strided DVE apply multiplies and
    the DMA's per-q contiguity both hold.
  - plane sum split: slots TREE_START..24 tree-added on DVE; slots
    0..TREE_START-1 fed individually into the out_proj PSUM accumulation on PE
    (PE/DVE load balance).
"""
import copy
import os
import numpy as np
from contextlib import ExitStack

import concourse.bacc as bacc
import concourse.tile as tile
import concourse.mybir as mybir
import concourse.bass_utils as bass_utils

F32 = mybir.dt.float32
F32R = mybir.dt.float32r
BF16 = mybir.dt.bfloat16
AF = mybir.ActivationFunctionType
OP = mybir.AluOpType

N_CORES = 8
NB, H, WD, C = 8, 64, 64, 256
G, GC, P = 16, 16, 9
T = H * WD              # 4096
Hp = 66                 # padded row width
MR = 2                  # margin rows
ROWS = Hp + 2 * MR      # 70
FS = ROWS * Hp
NCH = 8
TQ = 512
EPS = 1e-6
TREE_START = 16         # slots [TREE_START,25) summed on DVE; rest absorbed on PE

# tap order p: dx = p//3 - 1, dy = p%3 - 1
DX = [p // 3 - 1 for p in range(P)]
DY = [p % 3 - 1 for p in range(P)]


def slot_of(u_idx, v):
    """wball slot for plane (u_idx=u+2 in 0..4, v in -2..2)."""
    if v % 2 == 0:
        return 3 * u_idx + (v + 2) // 2
    return 15 + 2 * u_idx + (v + 1) // 2


# row-tile pairs for 144-row (g,p) tensors
PR = ((0, 0, 128), (1, 128, 16))


def _r(ap, spec, **kw):
    return ap.rearrange(spec, **kw)


def _win(padflat, r0, u, v, rows=8):
    start = (r0 + 1 + MR + u) * Hp + (1 + v)
    sl = padflat[:, start:start + rows * Hp]
    return _r(sl, "p (r c) -> p r c", c=Hp)[:, :, 0:WD]


def _strided(base, offs, dims):
    """Custom strided free-dim view of a [128, F] buffer."""
    v = base[:, offs:offs + 1]
    for _ in range(len(dims) - 1):
        v = v.unsqueeze(-1)
    a = copy.copy(v.ap)
    for i, (st, sz) in enumerate(dims):
        a[1 + i] = [st, sz]
    v2 = copy.copy(v)
    v2.ap = a
    return v2


def _win3(padflat, r0, u, v0, nv, odd, rows=8):
    start = (r0 + 1 + MR + u) * Hp + (1 + v0) - (1 if odd else 0)
    return _strided(padflat, start, [(2, nv), (Hp, rows), (1, WD)])


def _bcast_src(wsb, part0, nparts, sec, width):
    """[nparts part, (0,16), (1,width)] stride-0 replication source view."""
    src = wsb[part0:part0 + nparts, sec, 0:1]
    srcv = src.unsqueeze(1)
    aa = copy.copy(srcv.ap)
    aa[1] = [0, 16]
    aa[2] = [1, width]
    srcv2 = copy.copy(srcv)
    srcv2.ap = aa
    return srcv2


def build(debug=False):
    nc = bacc.Bacc("TRN2", target_bir_lowering=False, debug=False,
                   enable_asserts=True, num_devices=N_CORES)

    def din(name, shape, dt=F32):
        return nc.dram_tensor(name, list(shape), dt, kind="ExternalInput").ap()

    xT_d = din("xT", [C, T], BF16)
    dwdiag_d = din("dwdiag", [128, 18 * 128], BF16)
    inWT_d = din("inWT", [C, C], BF16)
    headWT_d = din("headWT", [C, 480], BF16)
    scand_d = din("scand", [128, 12 * 400], BF16)
    blk_d = din("blk", [144, 144], BF16)
    ones_d = din("ones", [128, 128], F32R)
    outWT_d = din("outWT", [C, C], BF16)
    bvec_d = din("bvec", [128, 16], F32)

    out_d = nc.dram_tensor("out", [C, T], F32, kind="ExternalOutput").ap()

    with tile.TileContext(nc) as tc, ExitStack() as ctx:
        consts = ctx.enter_context(tc.tile_pool(name="consts", bufs=1))
        big = ctx.enter_context(tc.tile_pool(name="big", bufs=1))
        ps = ctx.enter_context(tc.tile_pool(name="ps", bufs=1, space="PSUM"))

        # ---- PE warmup against HAM cold clock (runs during input DMA) ----
        if not os.environ.get("V2_NOWARM"):
            wrm = consts.tile([128, 128], BF16, name="wrm")
            nc.vector.memset(wrm, 0.01)
            wps = ps.tile([128, TQ], F32, tag="mm", bufs=2, name="wps")
            for _ in range(34):
                nc.tensor.matmul(wps[:, 0:128], wrm, wrm, start=True, stop=True)

        # ---- input: contiguous bf16 load, then on-chip scatter to padded ----
        xstack = ExitStack()
        xtp = xstack.enter_context(tc.tile_pool(name="xtp", bufs=1))
        work = xstack.enter_context(tc.tile_pool(name="work", bufs=1))
        xTc = xtp.tile([128, 2, T], BF16, name="xTc")
        for ct in range(2):
            for hh in range(2):
                nc.sync.dma_start(
                    out=xTc[:, ct, hh * 2048:(hh + 1) * 2048],
                    in_=xT_d[ct * 128:(ct + 1) * 128, hh * 2048:(hh + 1) * 2048])
        xTpad = [xtp.tile([128, FS], BF16, tag=f"xTpad{i}", name=f"xTpad{i}")
                 for i in range(2)]
        for ct in range(2):
            nc.vector.memset(xTpad[ct], 0.0)
        for ct in range(2):
            if os.environ.get("V2_NOSCATTER"):
                nc.vector.tensor_copy(
                    out=_win(xTpad[ct], 0, 0, 0, rows=H),
                    in_=_r(xTc[:, ct, :], "p (r c) -> p r c", c=WD))
            else:
                nc.sync.dma_start(
                    out=_win(xTpad[ct], 0, 0, 0, rows=H),
                    in_=_r(xTc[:, ct, :], "p (r c) -> p r c", c=WD))

        # ---- constants ----
        dwdiag = consts.tile([128, 18, 128], BF16, name="dwdiag")
        nc.sync.dma_start(out=dwdiag, in_=_r(dwdiag_d, "p (k m) -> p k m", m=128))
        inWT = consts.tile([128, 2, C], BF16, name="inWT")
        nc.sync.dma_start(out=inWT, in_=_r(inWT_d, "(k p) m -> p k m", p=128))
        ones = consts.tile([128, 128], F32R, name="ones")
        nc.sync.dma_start(out=ones, in_=ones_d)
        headWT = consts.tile([128, 2, 480], BF16, name="headWT")
        nc.sync.dma_start(out=headWT, in_=_r(headWT_d, "(k p) m -> p k m", p=128))
        scandW = consts.tile([128, 12, 400], BF16, name="scandW")
        nc.sync.dma_start(out=scandW, in_=_r(scand_d, "p (k m) -> p k m", m=400))
        blk = [consts.tile([128, 144], BF16, tag="bk0", name="blk0"),
               consts.tile([16, 144], BF16, tag="bk1", name="blk1")]
        nc.sync.dma_start(out=blk[0], in_=blk_d[0:128, :])
        nc.sync.dma_start(out=blk[1], in_=blk_d[128:144, :])
        outWT = consts.tile([128, 2, C], BF16, name="outWT")
        nc.sync.dma_start(out=outWT, in_=_r(outWT_d, "(k p) m -> p k m", p=128))
        bvec = consts.tile([128, 16], F32, name="bvec")
        nc.sync.dma_start(out=bvec, in_=bvec_d)
        dwB = [bvec[:, 0:1], bvec[:, 1:2]]
        lnG = [bvec[:, 2:3], bvec[:, 3:4]]
        lnB = [bvec[:, 4:5], bvec[:, 5:6]]
        inB = [bvec[:, 6:7], bvec[:, 7:8]]
        outB = [bvec[:, 8:9], bvec[:, 9:10]]
        headB = [(bvec[:, 10 + s:11 + s], bvec[0:16, 13 + s:14 + s]) for s in range(3)]
        epsT = consts.tile([128, 1], F32, name="epsT")
        nc.vector.memset(epsT, EPS)

        # ---- padded buffers ----
        xppad = [big.tile([128, FS], BF16, tag=f"xppad{i}", name=f"xppad{i}") for i in range(2)]
        xppod = [big.tile([128, FS], BF16, tag=f"xppod{i}", name=f"xppod{i}") for i in range(2)]
        x1 = [big.tile([128, T], BF16, tag=f"x1{i}", name=f"x1_{i}") for i in range(2)]
        nc.vector.memset(xppad[0], 0.0)
        nc.vector.memset(xppad[1], 0.0)
        nc.gpsimd.memset(xppod[0], 0.0)
        nc.gpsimd.memset(xppod[1], 0.0)

        with tc.tile_pool(name="psS", bufs=1, space="PSUM") as psS:
            # ---- S1: in_proj -> xppad (bf16) ----
            for tt in range(NCH):
                for mt in range(2):
                    pp = ps.tile([128, TQ], F32, tag="mm", bufs=2, name="pp")
                    for kt in range(2):
                        nc.tensor.matmul(pp, inWT[:, kt, mt * 128:(mt + 1) * 128],
                                         xTc[:, kt, tt * TQ:(tt + 1) * TQ],
                                         start=(kt == 0), stop=(kt == 1))
                    nc.scalar.activation(out=_win(xppad[mt], tt * 8, 0, 0),
                                         in_=_r(pp, "p (r c) -> p r c", c=WD),
                                         func=AF.Identity, bias=inB[mt], scale=1.0)

            for ct in range(2):
                nc.vector.tensor_copy(out=xppod[ct][:, 0:FS - 1], in_=xppad[ct][:, 1:FS])

            # ---- S2+S3: depthwise conv + LN + GELU -> x1 (bf16) ----
            for tt in range(NCH):
                ysb, y2sb = [], []
                for ct in range(2):
                    cp = ps.tile([128, TQ], F32, tag="mm", bufs=2, name="cp")
                    for tap in range(9):
                        ky, kx = tap // 3, tap % 3
                        rr = _win(xTpad[ct], tt * 8, ky - 1, kx - 1)
                        nc.tensor.matmul(cp, dwdiag[:, tap * 2 + ct, :],
                                         rr, start=(tap == 0), stop=(tap == 8))
                    y_ = work.tile([128, TQ], F32R, tag=f"ysb{ct}", name=f"ysb{ct}")
                    nc.scalar.activation(out=y_, in_=cp, func=AF.Identity, bias=dwB[ct], scale=1.0)
                    y2_ = work.tile([128, TQ], F32R, tag=f"y2sb{ct}", name=f"y2sb{ct}")
                    nc.scalar.activation(out=y2_, in_=cp, func=AF.Square, bias=dwB[ct], scale=1.0)
                    ysb.append(y_); y2sb.append(y2_)
                sp = psS.tile([128, TQ], F32, tag="s", bufs=1, name="sp")
                s2p = psS.tile([128, TQ], F32, tag="s2", bufs=1, name="s2p")
                for ct in range(2):
                    nc.tensor.matmul(sp, ones, ysb[ct], start=(ct == 0), stop=(ct == 1))
                    nc.tensor.matmul(s2p, ones, y2sb[ct], start=(ct == 0), stop=(ct == 1))
                mn = work.tile([128, TQ], F32, tag="lnm", name="lnm")
                nc.vector.tensor_scalar(out=mn, in0=sp, scalar1=1.0 / C, scalar2=None, op0=OP.mult)
                msq = work.tile([128, TQ], F32, tag="lnmsq", name="lnmsq")
                nc.vector.tensor_mul(out=msq, in0=mn, in1=mn)
                var = work.tile([128, TQ], F32, tag="lnvar", name="lnvar")
                nc.vector.scalar_tensor_tensor(out=var, in0=s2p, scalar=1.0 / C, in1=msq,
                                               op0=OP.mult, op1=OP.subtract)
                sd = work.tile([128, TQ], F32, tag="lnsd", name="lnsd")
                nc.scalar.activation(out=sd, in_=var, func=AF.Sqrt, bias=epsT, scale=1.0)
                rstd = work.tile([128, TQ], F32, tag="lnrstd", name="lnrstd")
                nc.vector.reciprocal_approx_fast(out=rstd, in_=sd)
                for ct in range(2):
                    t1 = work.tile([128, TQ], F32, tag="lnmsq", name="t1")
                    nc.vector.tensor_sub(out=t1, in0=ysb[ct].bitcast(F32), in1=mn)
                    t2 = work.tile([128, TQ], F32, tag="lnvar", name="t2")
                    nc.vector.tensor_mul(out=t2, in0=t1, in1=rstd)
                    nc.scalar.activation(out=x1[ct][:, tt * TQ:(tt + 1) * TQ], in_=t2,
                                         func=AF.Gelu, bias=lnB[ct], scale=lnG[ct])

        xstack.close()

        # ---- per-chunk pipeline ----
        qp = ctx.enter_context(tc.tile_pool(name="qp", bufs=1))
        sm = ctx.enter_context(tc.tile_pool(name="sm", bufs=2))
        psDen = ctx.enter_context(tc.tile_pool(name="psDen", bufs=1, space="PSUM"))
        psOut = ctx.enter_context(tc.tile_pool(name="psOut", bufs=1, space="PSUM"))

        def front(ch):
            """heads -> softmax -> tents -> packed q products for chunk ch."""
            x1sl = [x1[kt][:, ch * TQ:(ch + 1) * TQ] for kt in range(2)]
            # oxy[pi]: [_,2,512] offsets (x,y); em: exp(mask logits)
            oxy = [qp.tile([128, 2, TQ], BF16, tag="oxy0", bufs=1, name="oxy0"),
                   qp.tile([16, 2, TQ], BF16, tag="oxy1", bufs=1, name="oxy1")]
            em = [qp.tile([128, TQ], BF16, tag="em0", bufs=2, name="em0"),
                  qp.tile([16, TQ], BF16, tag="em1", bufs=2, name="em1")]
            for sec in range(3):
                for pi, m0, msz in PR:
                    hp = ps.tile([128, TQ], F32, tag="mm", bufs=2, name="hp")
                    for kt in range(2):
                        nc.tensor.matmul(hp[:msz], headWT[:, kt, sec * 160 + m0: sec * 160 + m0 + msz],
                                         x1sl[kt], start=(kt == 0), stop=(kt == 1))
                    if sec < 2:
                        nc.scalar.activation(out=oxy[pi][:msz, sec, :], in_=hp[:msz],
                                             func=AF.Identity, bias=headB[sec][pi], scale=1.0)
                    else:
                        nc.scalar.activation(out=em[pi][:msz], in_=hp[:msz],
                                             func=AF.Exp, bias=headB[sec][pi], scale=1.0)
            # softmax denominators per group, replicated to (g,p) rows
            den = [psDen.tile([128, TQ], F32, tag="dn0", bufs=1, name="dn0"),
                   psDen.tile([16, TQ], F32, tag="dn1", bufs=1, name="dn1")]
            for pii, (pi, m0, msz) in enumerate(PR):
                for kt, (kpi, k0, ksz) in enumerate(PR):
                    nc.tensor.matmul(den[pii][:msz], blk[kpi][:, m0:m0 + msz], em[kpi],
                                     start=(kt == 0), stop=(kt == 1))
            rs = [qp.tile([128, TQ], F32, tag="rs0", bufs=1, name="rs0"),
                  qp.tile([16, TQ], F32, tag="rs1", bufs=1, name="rs1")]
            for pi, m0, msz in PR:
                nc.vector.reciprocal_approx_fast(out=rs[pi][:msz], in_=den[pi][:msz])
            mt_ = [qp.tile([128, TQ], BF16, tag="mt0", bufs=1, name="mt0"),
                   qp.tile([16, TQ], BF16, tag="mt1", bufs=1, name="mt1")]
            for pi, m0, msz in PR:
                nc.vector.tensor_mul(out=mt_[pi][:msz], in0=em[pi][:msz], in1=rs[pi][:msz])
            # tents T[pi]: [_, cand(3), axis(2), t]; cand1 center negated (|o|-1)
            TT = [qp.tile([128, 3, 2, TQ], BF16, tag="T0", bufs=1, name="T0"),
                  qp.tile([16, 3, 2, TQ], BF16, tag="T1", bufs=1, name="T1")]
            for pi, m0, msz in PR:
                t = TT[pi]
                nc.vector.tensor_scalar(out=t[:msz, 0, :, :], in0=oxy[pi][:msz],
                                        scalar1=-1.0, scalar2=0.0, op0=OP.mult, op1=OP.max)
                nc.vector.tensor_scalar(out=t[:msz, 2, :, :], in0=oxy[pi][:msz],
                                        scalar1=0.0, scalar2=None, op0=OP.max)
                nc.vector.scalar_tensor_tensor(out=t[:msz, 1, :, :], in0=t[:msz, 0, :, :],
                                               scalar=1.0, in1=t[:msz, 2, :, :],
                                               op0=OP.subtract, op1=OP.add)
            # mty3[pi]: [_, cy(3), t] = mt * ty_cy  (stride-0 expand of mt)
            mty3 = [qp.tile([128, 3, TQ], BF16, tag="mty0", bufs=1, name="mty0"),
                    qp.tile([16, 3, TQ], BF16, tag="mty1", bufs=1, name="mty1")]
            for pi, m0, msz in PR:
                mtx = mt_[pi][:msz].unsqueeze(1)
                aa = copy.copy(mtx.ap)
                aa[1] = [0, 3]
                mtx3 = copy.copy(mtx)
                mtx3.ap = aa
                tyv = TT[pi][:msz, :, 1, :]
                nc.vector.tensor_mul(out=mty3[pi][:msz], in0=mtx3, in1=tyv)
            # q products: qm3[cy] [128, cx(3), t]; leftovers dense in qL/qL2
            qm3 = [qp.tile([128, 3, TQ], BF16, tag=f"qm{cy}", bufs=2, name=f"qm{cy}")
                   for cy in range(3)]
            txv0 = TT[0][:, :, 0, :]
            for cy in range(3):
                mv = mty3[0][:, cy, :].unsqueeze(1)
                aa = copy.copy(mv.ap)
                aa[1] = [0, 3]
                mv3 = copy.copy(mv)
                mv3.ap = aa
                nc.vector.tensor_mul(out=qm3[cy], in0=mv3, in1=txv0)
            qL = [qp.tile([128, TQ], BF16, tag="qLa", bufs=2, name="qLa"),
                  qp.tile([128, TQ], BF16, tag="qLb", bufs=2, name="qLb"),
                  qp.tile([16, TQ], BF16, tag="qLc", bufs=2, name="qLc")]
            for cand in range(9):
                cy, cx = cand // 3, cand % 3
                src0 = mty3[1][:, cy, :]
                src1 = TT[1][:, cx, 0, :]
                if cand < 8:
                    o = (cand % 4) * 32
                    nc.vector.tensor_mul(out=qL[cand // 4][o:o + 16], in0=src0, in1=src1)
                else:
                    nc.vector.tensor_mul(out=qL[2], in0=src0, in1=src1)
            return qm3, qL

        def backA(ch, qm3, qL):
            """scand matmuls -> wsb -> broadcast DMAs into wball."""
            wsb = qp.tile([128, 4, TQ], BF16, tag="wsb", bufs=2, name="wsb")
            for j in range(4):
                msz = 128 if j < 3 else 16
                wp = ps.tile([128, TQ], F32, tag="mm", bufs=2, name="wp")
                for k in range(12):
                    if k < 9:
                        rhs = qm3[k // 3][:, k % 3, :]
                        lhs = scandW[:, k, j * 128:j * 128 + msz]
                    elif k < 11:
                        rhs = qL[k - 9]
                        lhs = scandW[:, k, j * 128:j * 128 + msz]
                    else:
                        rhs = qL[2]
                        lhs = scandW[0:16, 11, j * 128:j * 128 + msz]
                    nc.tensor.matmul(wp[:msz], lhs, rhs, start=(k == 0), stop=(k == 11))
                nc.scalar.copy(out=wsb[:msz, j, :], in_=wp[:msz])
            # one shared tag, bufs=3: cycles over (chunk, ct) pairs
            wball = [qp.tile([128, 25 * TQ], BF16, tag="wball", bufs=2, name=f"wball{ct}")
                     for ct in range(2)]
            qeng = [nc.sync, nc.scalar]
            for ct in range(2):
                for q in range(8):
                    qeng[(ct * 9 + q) % 2].dma_start(
                        out=wball[ct][:, q * 3 * TQ:(q + 1) * 3 * TQ],
                        in_=_bcast_src(wsb, q * 16 + ct * 8, 8, 0, 3 * TQ))
                qeng[ct % 2].dma_start(
                    out=wball[ct][:, 24 * TQ:25 * TQ],
                    in_=_bcast_src(wsb, ct * 8, 8, 3, TQ))
            return wball

        def backB(ch, wball):
            """apply windows, partial tree, out_proj with absorbed planes."""
            for ct in range(2):
                wb = wball[ct]
                for iu in range(5):
                    u = iu - 2
                    wv = _strided(wb, (3 * iu) * TQ, [(TQ, 3), (64, 8), (1, WD)])
                    xv = _win3(xppod[ct], ch * 8, u, -2, 3, odd=True)
                    nc.vector.tensor_mul(out=wv, in0=wv, in1=xv)
                    wv2 = _strided(wb, (15 + 2 * iu) * TQ, [(TQ, 2), (64, 8), (1, WD)])
                    xv2 = _win3(xppad[ct], ch * 8, u, -1, 2, odd=False)
                    nc.vector.tensor_mul(out=wv2, in0=wv2, in1=xv2)
                # tree over slots [16,25): 9 slots -> acc at slot 16
                def wsl(a, b):
                    return wb[:, a * TQ:b * TQ]
                nc.vector.tensor_add(out=wsl(16, 20), in0=wsl(16, 20), in1=wsl(20, 24))
                nc.vector.tensor_add(out=wsl(16, 18), in0=wsl(16, 18), in1=wsl(18, 20))
                nc.vector.tensor_add(out=wsl(16, 17), in0=wsl(16, 17), in1=wsl(17, 18))
                nc.vector.tensor_add(out=wsl(16, 17), in0=wsl(16, 17), in1=wsl(24, 25))
            # out_proj: absorb slots 0..TREE_START-1, tree accs (slot 16) LAST
            if os.environ.get("V2_NOABSORB"):
                seq = [(0, 16), (1, 16)]
            else:
                seq = ([(ct, s) for ct in range(2) for s in range(TREE_START)]
                       + [(0, 16), (1, 16)])
            for mt in range(2):
                op_ = psOut.tile([128, TQ], F32, tag=f"op{mt}", bufs=2, name=f"op{mt}")
                nsteps = len(seq)
                step = 0
                for ct, s in seq:
                    lhs = outWT[:, ct, mt * 128:(mt + 1) * 128]
                    rhs = wball[ct][:, s * TQ:(s + 1) * TQ]
                    nc.tensor.matmul(op_, lhs, rhs,
                                     start=(step == 0), stop=(step == nsteps - 1))
                    step += 1
                o_ = sm.tile([128, TQ], F32, tag=f"osb{mt}", bufs=1, name=f"osb{mt}")
                nc.scalar.activation(out=o_, in_=op_, func=AF.Identity, bias=outB[mt], scale=1.0)
                nc.gpsimd.dma_start(out=out_d[mt * 128:(mt + 1) * 128, ch * TQ:(ch + 1) * TQ], in_=o_)

        for _ in range(2):
            nc.vector.memset(qp.tile([128, TQ], BF16, tag="qLa", bufs=2, name="qLa_i"), 0.0)
            nc.vector.memset(qp.tile([128, TQ], BF16, tag="qLb", bufs=2, name="qLb_i"), 0.0)

        pend = None
        for ch in range(NCH):
            if pend is not None:
                backB(ch - 1, pend)
            q = front(ch)
            pend = backA(ch, *q)
        backB(NCH - 1, pend)

    return nc


# ---------------- host side ----------------
_BUILT = {}


def _get_built(debug=False):
    key = bool(debug)
    if key not in _BUILT:
        nc = build(debug=debug)
        nc.compile()
        _BUILT[key] = nc
    return _BUILT[key]


def prep_weights(inputs):
    f32 = np.float32
    dw_w = np.asarray(inputs["dw_w"], f32)
    off_w = np.asarray(inputs["off_w"], f32)
    mask_w = np.asarray(inputs["mask_w"], f32)
    in_w = np.asarray(inputs["in_w"], f32)
    out_w = np.asarray(inputs["out_w"], f32)

    dwdiag = np.zeros((128, 18, 128), f32)
    cl = np.arange(128)
    for tap in range(9):
        ky, kx = tap // 3, tap % 3
        for ct in range(2):
            dwdiag[cl, tap * 2 + ct, cl] = dw_w[ct * 128:(ct + 1) * 128, 0, ky, kx]

    headWT = np.zeros((C, 480), f32)
    headB = np.zeros((480,), f32)
    off_b = np.asarray(inputs["off_b"], f32)
    mask_b = np.asarray(inputs["mask_b"], f32)
    for g in range(G):
        for p in range(P):
            r = g * P + p
            headWT[:, 0 * 160 + r] = off_w[g * 18 + p * 2 + 0]
            headWT[:, 1 * 160 + r] = off_w[g * 18 + p * 2 + 1]
            headWT[:, 2 * 160 + r] = mask_w[g * 9 + p]
            headB[0 * 160 + r] = off_b[g * 18 + p * 2 + 0]
            headB[1 * 160 + r] = off_b[g * 18 + p * 2 + 1]
            headB[2 * 160 + r] = mask_b[g * 9 + p]

    # scand: 11 contraction tiles x 512 outputs (4 sections x 128 rows)
    # output column for (slot s, group g): j*128 + q*16 + g  with
    #   s<24: q=s//3, j=s%3;  s==24: q=0, j=3
    scand = np.zeros((128, 12, 400), f32)
    for p in range(P):
        for cy in range(3):
            for cx in range(3):
                cand = cy * 3 + cx
                sgn = (-1.0 if cy == 1 else 1.0) * (-1.0 if cx == 1 else 1.0)
                u_idx = DY[p] + (cy - 1) + 2
                v = DX[p] + (cx - 1)
                s = slot_of(u_idx, v)
                q, j = (s // 3, s % 3) if s < 24 else (0, 3)
                for g in range(G):
                    gp = g * 9 + p
                    col = j * 128 + q * 16 + g if j < 3 else 384 + g
                    if gp < 128:
                        scand[gp, cand, col] = sgn
                    elif cand < 8:
                        scand[(cand % 4) * 32 + (gp - 128), 9 + cand // 4, col] = sgn
                    else:
                        scand[gp - 128, 11, col] = sgn

    blk = np.zeros((144, 144), f32)
    for g in range(G):
        blk[g * P:(g + 1) * P, g * P:(g + 1) * P] = 1.0

    bvec = np.zeros((128, 16), f32)
    def put2(col, v):
        bvec[:, col] = v[0:128]
        bvec[:, col + 1] = v[128:256]
    put2(0, np.asarray(inputs["dw_b"], f32))
    put2(2, np.asarray(inputs["ln_g"], f32))
    put2(4, np.asarray(inputs["ln_b"], f32))
    put2(6, np.asarray(inputs["in_b"], f32))
    put2(8, np.asarray(inputs["out_b"], f32))
    for s in range(3):
        bvec[:, 10 + s] = headB[s * 160: s * 160 + 128]
        bvec[0:16, 13 + s] = headB[s * 160 + 128: s * 160 + 144]

    import ml_dtypes
    tobf = lambda a: np.ascontiguousarray(a).astype(ml_dtypes.bfloat16)

    return {
        "dwdiag": tobf(dwdiag.reshape(128, 18 * 128)),
        "inWT": tobf(in_w.T),
        "headWT": tobf(headWT),
        "scand": tobf(scand.reshape(128, 12 * 400)),
        "blk": tobf(blk),
        "ones": np.ones((128, 128), f32),
        "outWT": tobf(out_w.T),
        "bvec": bvec,
    }


def kernel(**inputs):
    import ml_dtypes
    nc = _get_built(debug=False)
    wts = prep_weights(inputs)
    x = np.asarray(inputs["x"], np.float32)
    in_maps = []
    for n in range(N_CORES):
        m = dict(wts)
        m["xT"] = np.ascontiguousarray(x[n].reshape(T, C).T).astype(ml_dtypes.bfloat16)
        in_maps.append(m)
    res = bass_utils.run_bass_kernel_spmd(nc, in_maps, core_ids=list(range(N_CORES)))
    out = np.stack([np.ascontiguousarray(res.results[n]["out"].reshape(C, T).T).reshape(H, WD, C)
                    for n in range(N_CORES)])
    return out


# revision 16
# speedup vs baseline: 1.3279x; 1.3279x over previous
"""DCNv3 Trainium2 Bass kernel v2 — data-parallel over batch (1 image per core).

Structure (vs v1): channels-on-partitions [C=2x128, t=H*W]; spatial shifts are
free-dim AP offsets into zero-padded flat buffers.
  - in_proj / depthwise conv (diagonal matmuls) / LN (ones-matmul channel sums)
    / GELU on PE+ACT+DVE.  Input x uploaded bf16, contiguous DMA, on-chip
    scatter into the padded layout.
  - bilinear sampling as 5x5 dynamic local window: 25 weight planes W[plane,g,t]
    built by a selection matmul (scand) over packed tent products.
  - W group->channel broadcast via SBUF->SBUF DMA with a stride-0 source dim
    (src [8part,(0,16),(1,1536)] -> dst [128part,(1,1536)]), replacing v1's
    400 PE broadcast matmuls + ACT PSUM drains.
  - plane slot order: slots 3u+ci (ci: v=-2,0,2; odd buffer) for u=0..4, then
    15+2u+e (v=-1,+1; even buffer), so the 10 strided DVE apply multiplies and
    the DMA's per-q contiguity both hold.
  - plane sum split: slots TREE_START..24 tree-added on DVE; slots
    0..TREE_START-1 fed individually into the out_proj PSUM accumulation on PE
    (PE/DVE load balance).
"""
import copy
import os
import numpy as np
from contextlib import ExitStack

import concourse.bacc as bacc
import concourse.tile as tile
import concourse.mybir as mybir
import concourse.bass_utils as bass_utils

F32 = mybir.dt.float32
F32R = mybir.dt.float32r
BF16 = mybir.dt.bfloat16
AF = mybir.ActivationFunctionType
OP = mybir.AluOpType

N_CORES = 8
NB, H, WD, C = 8, 64, 64, 256
G, GC, P = 16, 16, 9
T = H * WD              # 4096
Hp = 66                 # padded row width
MR = 2                  # margin rows
ROWS = Hp + 2 * MR      # 70
FS = ROWS * Hp
NCH = 8
TQ = 512
EPS = 1e-6
TREE_START = 15         # slots [TREE_START,25) summed on DVE; rest absorbed on PE

# tap order p: dx = p//3 - 1, dy = p%3 - 1
DX = [p // 3 - 1 for p in range(P)]
DY = [p % 3 - 1 for p in range(P)]


def slot_of(u_idx, v):
    """wball slot for plane (u_idx=u+2 in 0..4, v in -2..2)."""
    if v % 2 == 0:
        return 3 * u_idx + (v + 2) // 2
    return 15 + 2 * u_idx + (v + 1) // 2


# row-tile pairs for 144-row (g,p) tensors
PR = ((0, 0, 128), (1, 128, 16))


def _r(ap, spec, **kw):
    return ap.rearrange(spec, **kw)


def _win(padflat, r0, u, v, rows=8):
    start = (r0 + 1 + MR + u) * Hp + (1 + v)
    sl = padflat[:, start:start + rows * Hp]
    return _r(sl, "p (r c) -> p r c", c=Hp)[:, :, 0:WD]


def _strided(base, offs, dims):
    """Custom strided free-dim view of a [128, F] buffer."""
    v = base[:, offs:offs + 1]
    for _ in range(len(dims) - 1):
        v = v.unsqueeze(-1)
    a = copy.copy(v.ap)
    for i, (st, sz) in enumerate(dims):
        a[1 + i] = [st, sz]
    v2 = copy.copy(v)
    v2.ap = a
    return v2


def _win3(padflat, r0, u, v0, nv, odd, rows=8):
    start = (r0 + 1 + MR + u) * Hp + (1 + v0) - (1 if odd else 0)
    return _strided(padflat, start, [(2, nv), (Hp, rows), (1, WD)])


def _bcast_src(wsb, part0, nparts, sec, width):
    """[nparts part, (0,16), (1,width)] stride-0 replication source view."""
    src = wsb[part0:part0 + nparts, sec, 0:1]
    srcv = src.unsqueeze(1)
    aa = copy.copy(srcv.ap)
    aa[1] = [0, 16]
    aa[2] = [1, width]
    srcv2 = copy.copy(srcv)
    srcv2.ap = aa
    return srcv2


def build(debug=False):
    nc = bacc.Bacc("TRN2", target_bir_lowering=False, debug=False,
                   enable_asserts=True, num_devices=N_CORES)

    def din(name, shape, dt=F32):
        return nc.dram_tensor(name, list(shape), dt, kind="ExternalInput").ap()

    xT_d = din("xT", [C, T], BF16)
    dwdiag_d = din("dwdiag", [128, 18 * 128], BF16)
    inWT_d = din("inWT", [C, C], BF16)
    headWT_d = din("headWT", [C, 480], BF16)
    scand_d = din("scand", [128, 12 * 400], BF16)
    blk_d = din("blk", [144, 144], BF16)
    ones_d = din("ones", [128, 128], F32R)
    outWT_d = din("outWT", [C, C], BF16)
    gsel_d = din("gsel", [128, 4 * C], BF16)
    bvec_d = din("bvec", [128, 16], F32)

    out_d = nc.dram_tensor("out", [C, T], F32, kind="ExternalOutput").ap()

    with tile.TileContext(nc) as tc, ExitStack() as ctx:
        consts = ctx.enter_context(tc.tile_pool(name="consts", bufs=1))
        big = ctx.enter_context(tc.tile_pool(name="big", bufs=1))
        ps = ctx.enter_context(tc.tile_pool(name="ps", bufs=1, space="PSUM"))

        # ---- PE warmup against HAM cold clock (runs during input DMA) ----
        if not os.environ.get("V2_NOWARM"):
            wrm = consts.tile([128, 128], BF16, name="wrm")
            nc.vector.memset(wrm, 0.01)
            wps = ps.tile([128, TQ], F32, tag="mm", bufs=2, name="wps")
            for _ in range(34):
                nc.tensor.matmul(wps[:, 0:128], wrm, wrm, start=True, stop=True)

        # ---- input: contiguous bf16 load, then on-chip scatter to padded ----
        xstack = ExitStack()
        xtp = xstack.enter_context(tc.tile_pool(name="xtp", bufs=1))
        work = xstack.enter_context(tc.tile_pool(name="work", bufs=1))
        xTc = xtp.tile([128, 2, T], BF16, name="xTc")
        for ct in range(2):
            for hh in range(2):
                nc.sync.dma_start(
                    out=xTc[:, ct, hh * 2048:(hh + 1) * 2048],
                    in_=xT_d[ct * 128:(ct + 1) * 128, hh * 2048:(hh + 1) * 2048])
        xTpad = [xtp.tile([128, FS], BF16, tag=f"xTpad{i}", name=f"xTpad{i}")
                 for i in range(2)]
        for ct in range(2):
            nc.vector.memset(xTpad[ct], 0.0)
        for ct in range(2):
            if os.environ.get("V2_NOSCATTER"):
                nc.vector.tensor_copy(
                    out=_win(xTpad[ct], 0, 0, 0, rows=H),
                    in_=_r(xTc[:, ct, :], "p (r c) -> p r c", c=WD))
            else:
                nc.sync.dma_start(
                    out=_win(xTpad[ct], 0, 0, 0, rows=H),
                    in_=_r(xTc[:, ct, :], "p (r c) -> p r c", c=WD))

        # ---- constants ----
        dwdiag = consts.tile([128, 18, 128], BF16, name="dwdiag")
        nc.sync.dma_start(out=dwdiag, in_=_r(dwdiag_d, "p (k m) -> p k m", m=128))
        inWT = consts.tile([128, 2, C], BF16, name="inWT")
        nc.sync.dma_start(out=inWT, in_=_r(inWT_d, "(k p) m -> p k m", p=128))
        ones = consts.tile([128, 128], F32R, name="ones")
        nc.sync.dma_start(out=ones, in_=ones_d)
        headWT = consts.tile([128, 2, 480], BF16, name="headWT")
        nc.sync.dma_start(out=headWT, in_=_r(headWT_d, "(k p) m -> p k m", p=128))
        scandW = consts.tile([128, 12, 400], BF16, name="scandW")
        nc.sync.dma_start(out=scandW, in_=_r(scand_d, "p (k m) -> p k m", m=400))
        blk = [consts.tile([128, 144], BF16, tag="bk0", name="blk0"),
               consts.tile([16, 144], BF16, tag="bk1", name="blk1")]
        nc.sync.dma_start(out=blk[0], in_=blk_d[0:128, :])
        nc.sync.dma_start(out=blk[1], in_=blk_d[128:144, :])
        outWT = consts.tile([128, 2, C], BF16, name="outWT")
        nc.sync.dma_start(out=outWT, in_=_r(outWT_d, "(k p) m -> p k m", p=128))
        gsel = consts.tile([128, 4, C], BF16, name="gsel")
        nc.sync.dma_start(out=gsel, in_=_r(gsel_d, "p (k m) -> p k m", m=C))
        bvec = consts.tile([128, 16], F32, name="bvec")
        nc.sync.dma_start(out=bvec, in_=bvec_d)
        dwB = [bvec[:, 0:1], bvec[:, 1:2]]
        lnG = [bvec[:, 2:3], bvec[:, 3:4]]
        lnB = [bvec[:, 4:5], bvec[:, 5:6]]
        inB = [bvec[:, 6:7], bvec[:, 7:8]]
        outB = [bvec[:, 8:9], bvec[:, 9:10]]
        headB = [(bvec[:, 10 + s:11 + s], bvec[0:16, 13 + s:14 + s]) for s in range(3)]
        epsT = consts.tile([128, 1], F32, name="epsT")
        nc.vector.memset(epsT, EPS)

        # ---- padded buffers ----
        xppad = [big.tile([128, FS], BF16, tag=f"xppad{i}", name=f"xppad{i}") for i in range(2)]
        xppod = [big.tile([128, FS], BF16, tag=f"xppod{i}", name=f"xppod{i}") for i in range(2)]
        x1 = [big.tile([128, T], BF16, tag=f"x1{i}", name=f"x1_{i}") for i in range(2)]
        nc.vector.memset(xppad[0], 0.0)
        nc.vector.memset(xppad[1], 0.0)
        nc.gpsimd.memset(xppod[0], 0.0)
        nc.gpsimd.memset(xppod[1], 0.0)

        with tc.tile_pool(name="psS", bufs=1, space="PSUM") as psS:
            # ---- S1: in_proj -> xppad (bf16) ----
            for tt in range(NCH):
                for mt in range(2):
                    pp = ps.tile([128, TQ], F32, tag="mm", bufs=2, name="pp")
                    for kt in range(2):
                        nc.tensor.matmul(pp, inWT[:, kt, mt * 128:(mt + 1) * 128],
                                         xTc[:, kt, tt * TQ:(tt + 1) * TQ],
                                         start=(kt == 0), stop=(kt == 1))
                    nc.scalar.activation(out=_win(xppad[mt], tt * 8, 0, 0),
                                         in_=_r(pp, "p (r c) -> p r c", c=WD),
                                         func=AF.Identity, bias=inB[mt], scale=1.0)

            for ct in range(2):
                nc.vector.tensor_copy(out=xppod[ct][:, 0:FS - 1], in_=xppad[ct][:, 1:FS])

            # ---- S2+S3: depthwise conv + LN + GELU -> x1 (bf16) ----
            for tt in range(NCH):
                ysb, y2sb = [], []
                for ct in range(2):
                    cp = ps.tile([128, TQ], F32, tag="mm", bufs=2, name="cp")
                    for tap in range(9):
                        ky, kx = tap // 3, tap % 3
                        rr = _win(xTpad[ct], tt * 8, ky - 1, kx - 1)
                        nc.tensor.matmul(cp, dwdiag[:, tap * 2 + ct, :],
                                         rr, start=(tap == 0), stop=(tap == 8))
                    y_ = work.tile([128, TQ], F32R, tag=f"ysb{ct}", name=f"ysb{ct}")
                    nc.scalar.activation(out=y_, in_=cp, func=AF.Identity, bias=dwB[ct], scale=1.0)
                    y2_ = work.tile([128, TQ], F32R, tag=f"y2sb{ct}", name=f"y2sb{ct}")
                    nc.scalar.activation(out=y2_, in_=cp, func=AF.Square, bias=dwB[ct], scale=1.0)
                    ysb.append(y_); y2sb.append(y2_)
                sp = psS.tile([128, TQ], F32, tag="s", bufs=1, name="sp")
                s2p = psS.tile([128, TQ], F32, tag="s2", bufs=1, name="s2p")
                for ct in range(2):
                    nc.tensor.matmul(sp, ones, ysb[ct], start=(ct == 0), stop=(ct == 1))
                    nc.tensor.matmul(s2p, ones, y2sb[ct], start=(ct == 0), stop=(ct == 1))
                mn = work.tile([128, TQ], F32, tag="lnm", name="lnm")
                nc.vector.tensor_scalar(out=mn, in0=sp, scalar1=1.0 / C, scalar2=None, op0=OP.mult)
                msq = work.tile([128, TQ], F32, tag="lnmsq", name="lnmsq")
                nc.vector.tensor_mul(out=msq, in0=mn, in1=mn)
                var = work.tile([128, TQ], F32, tag="lnvar", name="lnvar")
                nc.vector.scalar_tensor_tensor(out=var, in0=s2p, scalar=1.0 / C, in1=msq,
                                               op0=OP.mult, op1=OP.subtract)
                sd = work.tile([128, TQ], F32, tag="lnsd", name="lnsd")
                nc.scalar.activation(out=sd, in_=var, func=AF.Sqrt, bias=epsT, scale=1.0)
                rstd = work.tile([128, TQ], F32, tag="lnrstd", name="lnrstd")
                nc.vector.reciprocal_approx_fast(out=rstd, in_=sd)
                for ct in range(2):
                    t1 = work.tile([128, TQ], F32, tag="lnmsq", name="t1")
                    nc.vector.tensor_sub(out=t1, in0=ysb[ct].bitcast(F32), in1=mn)
                    t2 = work.tile([128, TQ], F32, tag="lnvar", name="t2")
                    nc.vector.tensor_mul(out=t2, in0=t1, in1=rstd)
                    nc.scalar.activation(out=x1[ct][:, tt * TQ:(tt + 1) * TQ], in_=t2,
                                         func=AF.Gelu, bias=lnB[ct], scale=lnG[ct])

        xstack.close()

        # ---- per-chunk pipeline ----
        qp = ctx.enter_context(tc.tile_pool(name="qp", bufs=1))
        sm = ctx.enter_context(tc.tile_pool(name="sm", bufs=2))
        psDen = ctx.enter_context(tc.tile_pool(name="psDen", bufs=1, space="PSUM"))
        psOut = ctx.enter_context(tc.tile_pool(name="psOut", bufs=1, space="PSUM"))
        psW = ctx.enter_context(tc.tile_pool(name="psW", bufs=1, space="PSUM"))

        def front(ch):
            """heads -> softmax -> tents -> packed q products for chunk ch."""
            x1sl = [x1[kt][:, ch * TQ:(ch + 1) * TQ] for kt in range(2)]
            # oxy[pi]: [_,2,512] offsets (x,y); em: exp(mask logits)
            oxy = [qp.tile([128, 2, TQ], BF16, tag="oxy0", bufs=1, name="oxy0"),
                   qp.tile([16, 2, TQ], BF16, tag="oxy1", bufs=1, name="oxy1")]
            em = [qp.tile([128, TQ], BF16, tag="em0", bufs=2, name="em0"),
                  qp.tile([16, TQ], BF16, tag="em1", bufs=2, name="em1")]
            for sec in range(3):
                for pi, m0, msz in PR:
                    hp = ps.tile([128, TQ], F32, tag="mm", bufs=2, name="hp")
                    for kt in range(2):
                        nc.tensor.matmul(hp[:msz], headWT[:, kt, sec * 160 + m0: sec * 160 + m0 + msz],
                                         x1sl[kt], start=(kt == 0), stop=(kt == 1))
                    if sec < 2:
                        nc.scalar.activation(out=oxy[pi][:msz, sec, :], in_=hp[:msz],
                                             func=AF.Identity, bias=headB[sec][pi], scale=1.0)
                    else:
                        nc.scalar.activation(out=em[pi][:msz], in_=hp[:msz],
                                             func=AF.Exp, bias=headB[sec][pi], scale=1.0)
            # softmax denominators per group, replicated to (g,p) rows
            den = [psDen.tile([128, TQ], F32, tag="dn0", bufs=1, name="dn0"),
                   psDen.tile([16, TQ], F32, tag="dn1", bufs=1, name="dn1")]
            for pii, (pi, m0, msz) in enumerate(PR):
                for kt, (kpi, k0, ksz) in enumerate(PR):
                    nc.tensor.matmul(den[pii][:msz], blk[kpi][:, m0:m0 + msz], em[kpi],
                                     start=(kt == 0), stop=(kt == 1))
            rs = [qp.tile([128, TQ], F32, tag="rs0", bufs=1, name="rs0"),
                  qp.tile([16, TQ], F32, tag="rs1", bufs=1, name="rs1")]
            for pi, m0, msz in PR:
                nc.vector.reciprocal_approx_fast(out=rs[pi][:msz], in_=den[pi][:msz])
            mt_ = [qp.tile([128, TQ], BF16, tag="mt0", bufs=1, name="mt0"),
                   qp.tile([16, TQ], BF16, tag="mt1", bufs=1, name="mt1")]
            for pi, m0, msz in PR:
                nc.vector.tensor_mul(out=mt_[pi][:msz], in0=em[pi][:msz], in1=rs[pi][:msz])
            # tents T[pi]: [_, cand(3), axis(2), t]; cand1 center negated (|o|-1)
            TT = [qp.tile([128, 3, 2, TQ], BF16, tag="T0", bufs=1, name="T0"),
                  qp.tile([16, 3, 2, TQ], BF16, tag="T1", bufs=1, name="T1")]
            for pi, m0, msz in PR:
                t = TT[pi]
                nc.vector.tensor_scalar(out=t[:msz, 0, :, :], in0=oxy[pi][:msz],
                                        scalar1=-1.0, scalar2=0.0, op0=OP.mult, op1=OP.max)
                nc.vector.tensor_scalar(out=t[:msz, 2, :, :], in0=oxy[pi][:msz],
                                        scalar1=0.0, scalar2=None, op0=OP.max)
                nc.vector.scalar_tensor_tensor(out=t[:msz, 1, :, :], in0=t[:msz, 0, :, :],
                                               scalar=1.0, in1=t[:msz, 2, :, :],
                                               op0=OP.subtract, op1=OP.add)
            # mty3[pi]: [_, cy(3), t] = mt * ty_cy  (stride-0 expand of mt)
            mty3 = [qp.tile([128, 3, TQ], BF16, tag="mty0", bufs=1, name="mty0"),
                    qp.tile([16, 3, TQ], BF16, tag="mty1", bufs=1, name="mty1")]
            for pi, m0, msz in PR:
                mtx = mt_[pi][:msz].unsqueeze(1)
                aa = copy.copy(mtx.ap)
                aa[1] = [0, 3]
                mtx3 = copy.copy(mtx)
                mtx3.ap = aa
                tyv = TT[pi][:msz, :, 1, :]
                nc.vector.tensor_mul(out=mty3[pi][:msz], in0=mtx3, in1=tyv)
            # q products: qm3[cy] [128, cx(3), t]; leftovers dense in qL/qL2
            qm3 = [qp.tile([128, 3, TQ], BF16, tag=f"qm{cy}", bufs=2, name=f"qm{cy}")
                   for cy in range(3)]
            txv0 = TT[0][:, :, 0, :]
            for cy in range(3):
                mv = mty3[0][:, cy, :].unsqueeze(1)
                aa = copy.copy(mv.ap)
                aa[1] = [0, 3]
                mv3 = copy.copy(mv)
                mv3.ap = aa
                nc.vector.tensor_mul(out=qm3[cy], in0=mv3, in1=txv0)
            qL = [qp.tile([128, TQ], BF16, tag="qLa", bufs=2, name="qLa"),
                  qp.tile([128, TQ], BF16, tag="qLb", bufs=2, name="qLb"),
                  qp.tile([16, TQ], BF16, tag="qLc", bufs=2, name="qLc")]
            for cand in range(9):
                cy, cx = cand // 3, cand % 3
                src0 = mty3[1][:, cy, :]
                src1 = TT[1][:, cx, 0, :]
                if cand < 8:
                    o = (cand % 4) * 32
                    nc.vector.tensor_mul(out=qL[cand // 4][o:o + 16], in0=src0, in1=src1)
                else:
                    nc.vector.tensor_mul(out=qL[2], in0=src0, in1=src1)
            return qm3, qL

        def backA(ch, qm3, qL):
            """scand matmuls -> wsb -> broadcast DMAs into wball."""
            wsb = qp.tile([128, 4, TQ], BF16, tag="wsb", bufs=2, name="wsb")
            for j in range(4):
                msz = 128 if j < 3 else 16
                wp = ps.tile([128, TQ], F32, tag="mm", bufs=2, name="wp")
                for k in range(12):
                    if k < 9:
                        rhs = qm3[k // 3][:, k % 3, :]
                        lhs = scandW[:, k, j * 128:j * 128 + msz]
                    elif k < 11:
                        rhs = qL[k - 9]
                        lhs = scandW[:, k, j * 128:j * 128 + msz]
                    else:
                        rhs = qL[2]
                        lhs = scandW[0:16, 11, j * 128:j * 128 + msz]
                    nc.tensor.matmul(wp[:msz], lhs, rhs, start=(k == 0), stop=(k == 11))
                nc.scalar.copy(out=wsb[:msz, j, :], in_=wp[:msz])
            # one shared tag, bufs=3: cycles over (chunk, ct) pairs
            wball = [qp.tile([128, 25 * TQ], BF16, tag="wball", bufs=2, name=f"wball{ct}")
                     for ct in range(2)]
            qeng = [nc.sync, nc.gpsimd]
            for ct in range(2):
                for q in range(5):
                    qeng[(ct * 5 + q) % 2].dma_start(
                        out=wball[ct][:, q * 3 * TQ:(q + 1) * 3 * TQ],
                        in_=_bcast_src(wsb, q * 16 + ct * 8, 8, 0, 3 * TQ))
            # slots 15..24 via PE selection matmul + ACT drain
            # slot s<24: (q,j) = (s//3, s%3), gsel k-index q-5; slot 24: sec 3, gsel 3
            for ct in range(2):
                for s in range(15, 25):
                    if s < 24:
                        q, j = s // 3, s % 3
                        gi = q - 5
                    else:
                        j, gi = 3, 3
                    wbt = psW.tile([128, TQ], F32, tag="wb", bufs=2, name="wb")
                    ksz = 128 if s < 24 else 16
                    nc.tensor.matmul(wbt, gsel[0:ksz, gi, ct * 128:(ct + 1) * 128],
                                     wsb[0:ksz, j, :], start=True, stop=True)
                    nc.scalar.copy(out=wball[ct][:, s * TQ:(s + 1) * TQ], in_=wbt)
            return wball

        def backB(ch, wball):
            """apply windows, partial tree, out_proj with absorbed planes."""
            for ct in range(2):
                wb = wball[ct]
                for iu in range(5):
                    u = iu - 2
                    wv = _strided(wb, (3 * iu) * TQ, [(TQ, 3), (64, 8), (1, WD)])
                    xv = _win3(xppod[ct], ch * 8, u, -2, 3, odd=True)
                    nc.vector.tensor_mul(out=wv, in0=wv, in1=xv)
                    wv2 = _strided(wb, (15 + 2 * iu) * TQ, [(TQ, 2), (64, 8), (1, WD)])
                    xv2 = _win3(xppad[ct], ch * 8, u, -1, 2, odd=False)
                    nc.vector.tensor_mul(out=wv2, in0=wv2, in1=xv2)
                # tree over slots [16,25): 9 slots -> acc at slot 16
                def wsl(a, b):
                    return wb[:, a * TQ:b * TQ]
                nc.vector.tensor_add(out=wsl(15, 20), in0=wsl(15, 20), in1=wsl(20, 25))
                nc.vector.tensor_add(out=wsl(15, 17), in0=wsl(15, 17), in1=wsl(17, 19))
                nc.vector.tensor_add(out=wsl(15, 16), in0=wsl(15, 16), in1=wsl(16, 17))
                nc.vector.tensor_add(out=wsl(15, 16), in0=wsl(15, 16), in1=wsl(19, 20))
            # out_proj: absorb slots 0..TREE_START-1, tree accs (slot 16) LAST
            if os.environ.get("V2_NOABSORB"):
                seq = [(0, 16), (1, 16)]
            else:
                seq = ([(ct, s) for ct in range(2) for s in range(TREE_START)]
                       + [(0, 15), (1, 15)])
            for mt in range(2):
                op_ = psOut.tile([128, TQ], F32, tag="op", bufs=2, name="op")
                nsteps = len(seq)
                step = 0
                for ct, s in seq:
                    lhs = outWT[:, ct, mt * 128:(mt + 1) * 128]
                    rhs = wball[ct][:, s * TQ:(s + 1) * TQ]
                    nc.tensor.matmul(op_, lhs, rhs,
                                     start=(step == 0), stop=(step == nsteps - 1))
                    step += 1
                o_ = sm.tile([128, TQ], F32, tag=f"osb{mt}", bufs=1, name=f"osb{mt}")
                nc.scalar.activation(out=o_, in_=op_, func=AF.Identity, bias=outB[mt], scale=1.0)
                nc.gpsimd.dma_start(out=out_d[mt * 128:(mt + 1) * 128, ch * TQ:(ch + 1) * TQ], in_=o_)

        for _ in range(2):
            nc.vector.memset(qp.tile([128, TQ], BF16, tag="qLa", bufs=2, name="qLa_i"), 0.0)
            nc.vector.memset(qp.tile([128, TQ], BF16, tag="qLb", bufs=2, name="qLb_i"), 0.0)

        pend = None
        for ch in range(NCH):
            if pend is not None:
                backB(ch - 1, pend)
            q = front(ch)
            pend = backA(ch, *q)
        backB(NCH - 1, pend)

    return nc


# ---------------- host side ----------------
_BUILT = {}


def _get_built(debug=False):
    key = bool(debug)
    if key not in _BUILT:
        nc = build(debug=debug)
        nc.compile()
        _BUILT[key] = nc
    return _BUILT[key]


def prep_weights(inputs):
    f32 = np.float32
    dw_w = np.asarray(inputs["dw_w"], f32)
    off_w = np.asarray(inputs["off_w"], f32)
    mask_w = np.asarray(inputs["mask_w"], f32)
    in_w = np.asarray(inputs["in_w"], f32)
    out_w = np.asarray(inputs["out_w"], f32)

    dwdiag = np.zeros((128, 18, 128), f32)
    cl = np.arange(128)
    for tap in range(9):
        ky, kx = tap // 3, tap % 3
        for ct in range(2):
            dwdiag[cl, tap * 2 + ct, cl] = dw_w[ct * 128:(ct + 1) * 128, 0, ky, kx]

    headWT = np.zeros((C, 480), f32)
    headB = np.zeros((480,), f32)
    off_b = np.asarray(inputs["off_b"], f32)
    mask_b = np.asarray(inputs["mask_b"], f32)
    for g in range(G):
        for p in range(P):
            r = g * P + p
            headWT[:, 0 * 160 + r] = off_w[g * 18 + p * 2 + 0]
            headWT[:, 1 * 160 + r] = off_w[g * 18 + p * 2 + 1]
            headWT[:, 2 * 160 + r] = mask_w[g * 9 + p]
            headB[0 * 160 + r] = off_b[g * 18 + p * 2 + 0]
            headB[1 * 160 + r] = off_b[g * 18 + p * 2 + 1]
            headB[2 * 160 + r] = mask_b[g * 9 + p]

    # scand: 11 contraction tiles x 512 outputs (4 sections x 128 rows)
    # output column for (slot s, group g): j*128 + q*16 + g  with
    #   s<24: q=s//3, j=s%3;  s==24: q=0, j=3
    scand = np.zeros((128, 12, 400), f32)
    for p in range(P):
        for cy in range(3):
            for cx in range(3):
                cand = cy * 3 + cx
                sgn = (-1.0 if cy == 1 else 1.0) * (-1.0 if cx == 1 else 1.0)
                u_idx = DY[p] + (cy - 1) + 2
                v = DX[p] + (cx - 1)
                s = slot_of(u_idx, v)
                q, j = (s // 3, s % 3) if s < 24 else (0, 3)
                for g in range(G):
                    gp = g * 9 + p
                    col = j * 128 + q * 16 + g if j < 3 else 384 + g
                    if gp < 128:
                        scand[gp, cand, col] = sgn
                    elif cand < 8:
                        scand[(cand % 4) * 32 + (gp - 128), 9 + cand // 4, col] = sgn
                    else:
                        scand[gp - 128, 11, col] = sgn

    blk = np.zeros((144, 144), f32)
    for g in range(G):
        blk[g * P:(g + 1) * P, g * P:(g + 1) * P] = 1.0

    bvec = np.zeros((128, 16), f32)
    def put2(col, v):
        bvec[:, col] = v[0:128]
        bvec[:, col + 1] = v[128:256]
    put2(0, np.asarray(inputs["dw_b"], f32))
    put2(2, np.asarray(inputs["ln_g"], f32))
    put2(4, np.asarray(inputs["ln_b"], f32))
    put2(6, np.asarray(inputs["in_b"], f32))
    put2(8, np.asarray(inputs["out_b"], f32))
    for s in range(3):
        bvec[:, 10 + s] = headB[s * 160: s * 160 + 128]
        bvec[0:16, 13 + s] = headB[s * 160 + 128: s * 160 + 144]

    gsel = np.zeros((128, 4, C), f32)
    for gi in range(4):
        q = gi + 5 if gi < 3 else 0
        for c in range(C):
            ct, cl = c // 128, c % 128
            gsel[q * 16 + ct * 8 + cl // 16, gi, c] = 1.0

    import ml_dtypes
    tobf = lambda a: np.ascontiguousarray(a).astype(ml_dtypes.bfloat16)

    return {
        "dwdiag": tobf(dwdiag.reshape(128, 18 * 128)),
        "inWT": tobf(in_w.T),
        "headWT": tobf(headWT),
        "scand": tobf(scand.reshape(128, 12 * 400)),
        "blk": tobf(blk),
        "ones": np.ones((128, 128), f32),
        "outWT": tobf(out_w.T),
        "gsel": tobf(gsel.reshape(128, 4 * C)),
        "bvec": bvec,
    }


def kernel(**inputs):
    import ml_dtypes
    nc = _get_built(debug=False)
    wts = prep_weights(inputs)
    x = np.asarray(inputs["x"], np.float32)
    in_maps = []
    for n in range(N_CORES):
        m = dict(wts)
        m["xT"] = np.ascontiguousarray(x[n].reshape(T, C).T).astype(ml_dtypes.bfloat16)
        in_maps.append(m)
    res = bass_utils.run_bass_kernel_spmd(nc, in_maps, core_ids=list(range(N_CORES)))
    out = np.stack([np.ascontiguousarray(res.results[n]["out"].reshape(C, T).T).reshape(H, WD, C)
                    for n in range(N_CORES)])
    return out


# revision 17
# speedup vs baseline: 1.6389x; 1.2342x over previous
"""DCNv3 Trainium2 Bass kernel v2 — data-parallel over batch (1 image per core).

Structure (vs v1): channels-on-partitions [C=2x128, t=H*W]; spatial shifts are
free-dim AP offsets into zero-padded flat buffers.
  - in_proj / depthwise conv (diagonal matmuls) / LN (ones-matmul channel sums)
    / GELU on PE+ACT+DVE.  Input x uploaded bf16, contiguous DMA, on-chip
    scatter into the padded layout.
  - bilinear sampling as 5x5 dynamic local window: 25 weight planes W[plane,g,t]
    built by a selection matmul (scand) over packed tent products.
  - W group->channel broadcast via SBUF->SBUF DMA with a stride-0 source dim
    (src [8part,(0,16),(1,1536)] -> dst [128part,(1,1536)]), replacing v1's
    400 PE broadcast matmuls + ACT PSUM drains.
  - plane slot order: slots 3u+ci (ci: v=-2,0,2; odd buffer) for u=0..4, then
    15+2u+e (v=-1,+1; even buffer), so the 10 strided DVE apply multiplies and
    the DMA's per-q contiguity both hold.
  - plane sum split: slots TREE_START..24 tree-added on DVE; slots
    0..TREE_START-1 fed individually into the out_proj PSUM accumulation on PE
    (PE/DVE load balance).
"""
import copy
import os
import numpy as np
from contextlib import ExitStack

import concourse.bacc as bacc
import concourse.tile as tile
import concourse.mybir as mybir
import concourse.bass_utils as bass_utils

F32 = mybir.dt.float32
F32R = mybir.dt.float32r
BF16 = mybir.dt.bfloat16
AF = mybir.ActivationFunctionType
OP = mybir.AluOpType

N_CORES = 8
NB, H, WD, C = 8, 64, 64, 256
G, GC, P = 16, 16, 9
T = H * WD              # 4096
Hp = 66                 # padded row width
MR = 2                  # margin rows
ROWS = Hp + 2 * MR      # 70
FS = ROWS * Hp
NCH = 8
TQ = 512
EPS = 1e-6
TREE_START = 13         # slots [TREE_START,25) summed on DVE; rest absorbed on PE

# tap order p: dx = p//3 - 1, dy = p%3 - 1
DX = [p // 3 - 1 for p in range(P)]
DY = [p % 3 - 1 for p in range(P)]


def slot_of(u_idx, v):
    """wball slot for plane (u_idx=u+2 in 0..4, v in -2..2)."""
    if v % 2 == 0:
        return 3 * u_idx + (v + 2) // 2
    return 15 + 2 * u_idx + (v + 1) // 2


# row-tile pairs for 144-row (g,p) tensors
PR = ((0, 0, 128), (1, 128, 16))


def _r(ap, spec, **kw):
    return ap.rearrange(spec, **kw)


def _win(padflat, r0, u, v, rows=8):
    start = (r0 + 1 + MR + u) * Hp + (1 + v)
    sl = padflat[:, start:start + rows * Hp]
    return _r(sl, "p (r c) -> p r c", c=Hp)[:, :, 0:WD]


def _strided(base, offs, dims):
    """Custom strided free-dim view of a [128, F] buffer."""
    v = base[:, offs:offs + 1]
    for _ in range(len(dims) - 1):
        v = v.unsqueeze(-1)
    a = copy.copy(v.ap)
    for i, (st, sz) in enumerate(dims):
        a[1 + i] = [st, sz]
    v2 = copy.copy(v)
    v2.ap = a
    return v2


def _win3(padflat, r0, u, v0, nv, odd, rows=8):
    start = (r0 + 1 + MR + u) * Hp + (1 + v0) - (1 if odd else 0)
    return _strided(padflat, start, [(2, nv), (Hp, rows), (1, WD)])


def _bcast_src(wsb, part0, nparts, sec, width):
    """[nparts part, (0,16), (1,width)] stride-0 replication source view."""
    src = wsb[part0:part0 + nparts, sec, 0:1]
    srcv = src.unsqueeze(1)
    aa = copy.copy(srcv.ap)
    aa[1] = [0, 16]
    aa[2] = [1, width]
    srcv2 = copy.copy(srcv)
    srcv2.ap = aa
    return srcv2


def build(debug=False):
    nc = bacc.Bacc("TRN2", target_bir_lowering=False, debug=False,
                   enable_asserts=True, num_devices=N_CORES)

    def din(name, shape, dt=F32):
        return nc.dram_tensor(name, list(shape), dt, kind="ExternalInput").ap()

    xT_d = din("xT", [C, T], BF16)
    dwdiag_d = din("dwdiag", [128, 18 * 128], BF16)
    inWT_d = din("inWT", [C, C], BF16)
    headWT_d = din("headWT", [C, 480], BF16)
    scand_d = din("scand", [128, 12 * 400], BF16)
    blk_d = din("blk", [144, 144], BF16)
    ones_d = din("ones", [128, 128], F32R)
    outWT_d = din("outWT", [C, C], BF16)
    gsel_d = din("gsel", [128, 9 * C], BF16)
    bvec_d = din("bvec", [128, 16], F32)

    out_d = nc.dram_tensor("out", [C, T], F32, kind="ExternalOutput").ap()

    with tile.TileContext(nc) as tc, ExitStack() as ctx:
        consts = ctx.enter_context(tc.tile_pool(name="consts", bufs=1))
        big = ctx.enter_context(tc.tile_pool(name="big", bufs=1))
        ps = ctx.enter_context(tc.tile_pool(name="ps", bufs=1, space="PSUM"))

        # ---- PE warmup against HAM cold clock (runs during input DMA) ----
        if not os.environ.get("V2_NOWARM"):
            wrm = consts.tile([128, 128], BF16, name="wrm")
            nc.vector.memset(wrm, 0.01)
            wps = ps.tile([128, TQ], F32, tag="mm", bufs=2, name="wps")
            for _ in range(34):
                nc.tensor.matmul(wps[:, 0:128], wrm, wrm, start=True, stop=True)

        # ---- input: contiguous bf16 load, then on-chip scatter to padded ----
        xstack = ExitStack()
        xtp = xstack.enter_context(tc.tile_pool(name="xtp", bufs=1))
        work = xstack.enter_context(tc.tile_pool(name="work", bufs=1))
        xTc = xtp.tile([128, 2, T], BF16, name="xTc")
        for ct in range(2):
            for hh in range(2):
                nc.sync.dma_start(
                    out=xTc[:, ct, hh * 2048:(hh + 1) * 2048],
                    in_=xT_d[ct * 128:(ct + 1) * 128, hh * 2048:(hh + 1) * 2048])
        xTpad = [xtp.tile([128, FS], BF16, tag=f"xTpad{i}", name=f"xTpad{i}")
                 for i in range(2)]
        for ct in range(2):
            nc.vector.memset(xTpad[ct], 0.0)
        for ct in range(2):
            if os.environ.get("V2_NOSCATTER"):
                nc.vector.tensor_copy(
                    out=_win(xTpad[ct], 0, 0, 0, rows=H),
                    in_=_r(xTc[:, ct, :], "p (r c) -> p r c", c=WD))
            else:
                nc.sync.dma_start(
                    out=_win(xTpad[ct], 0, 0, 0, rows=H),
                    in_=_r(xTc[:, ct, :], "p (r c) -> p r c", c=WD))

        # ---- constants ----
        dwdiag = consts.tile([128, 18, 128], BF16, name="dwdiag")
        nc.sync.dma_start(out=dwdiag, in_=_r(dwdiag_d, "p (k m) -> p k m", m=128))
        inWT = consts.tile([128, 2, C], BF16, name="inWT")
        nc.sync.dma_start(out=inWT, in_=_r(inWT_d, "(k p) m -> p k m", p=128))
        ones = consts.tile([128, 128], F32R, name="ones")
        nc.sync.dma_start(out=ones, in_=ones_d)
        headWT = consts.tile([128, 2, 480], BF16, name="headWT")
        nc.sync.dma_start(out=headWT, in_=_r(headWT_d, "(k p) m -> p k m", p=128))
        scandW = consts.tile([128, 12, 400], BF16, name="scandW")
        nc.sync.dma_start(out=scandW, in_=_r(scand_d, "p (k m) -> p k m", m=400))
        blk = [consts.tile([128, 144], BF16, tag="bk0", name="blk0"),
               consts.tile([16, 144], BF16, tag="bk1", name="blk1")]
        nc.sync.dma_start(out=blk[0], in_=blk_d[0:128, :])
        nc.sync.dma_start(out=blk[1], in_=blk_d[128:144, :])
        outWT = consts.tile([128, 2, C], BF16, name="outWT")
        nc.sync.dma_start(out=outWT, in_=_r(outWT_d, "(k p) m -> p k m", p=128))
        gsel = consts.tile([128, 9, C], BF16, name="gsel")
        nc.sync.dma_start(out=gsel, in_=_r(gsel_d, "p (k m) -> p k m", m=C))
        bvec = consts.tile([128, 16], F32, name="bvec")
        nc.sync.dma_start(out=bvec, in_=bvec_d)
        dwB = [bvec[:, 0:1], bvec[:, 1:2]]
        lnG = [bvec[:, 2:3], bvec[:, 3:4]]
        lnB = [bvec[:, 4:5], bvec[:, 5:6]]
        inB = [bvec[:, 6:7], bvec[:, 7:8]]
        outB = [bvec[:, 8:9], bvec[:, 9:10]]
        headB = [(bvec[:, 10 + s:11 + s], bvec[0:16, 13 + s:14 + s]) for s in range(3)]
        epsT = consts.tile([128, 1], F32, name="epsT")
        nc.vector.memset(epsT, EPS)

        # ---- padded buffers ----
        xppad = [big.tile([128, FS], BF16, tag=f"xppad{i}", name=f"xppad{i}") for i in range(2)]
        xppod = [big.tile([128, FS], BF16, tag=f"xppod{i}", name=f"xppod{i}") for i in range(2)]
        x1 = [big.tile([128, T], BF16, tag=f"x1{i}", name=f"x1_{i}") for i in range(2)]
        nc.vector.memset(xppad[0], 0.0)
        nc.vector.memset(xppad[1], 0.0)
        nc.gpsimd.memset(xppod[0], 0.0)
        nc.gpsimd.memset(xppod[1], 0.0)

        with tc.tile_pool(name="psS", bufs=1, space="PSUM") as psS:
            # ---- S1: in_proj -> xppad (bf16) ----
            for tt in range(NCH):
                for mt in range(2):
                    pp = ps.tile([128, TQ], F32, tag="mm", bufs=2, name="pp")
                    for kt in range(2):
                        nc.tensor.matmul(pp, inWT[:, kt, mt * 128:(mt + 1) * 128],
                                         xTc[:, kt, tt * TQ:(tt + 1) * TQ],
                                         start=(kt == 0), stop=(kt == 1))
                    nc.scalar.activation(out=_win(xppad[mt], tt * 8, 0, 0),
                                         in_=_r(pp, "p (r c) -> p r c", c=WD),
                                         func=AF.Identity, bias=inB[mt], scale=1.0)

            for ct in range(2):
                nc.vector.tensor_copy(out=xppod[ct][:, 0:FS - 1], in_=xppad[ct][:, 1:FS])

            # ---- S2+S3: depthwise conv + LN + GELU -> x1 (bf16) ----
            for tt in range(NCH):
                ysb, y2sb = [], []
                for ct in range(2):
                    cp = ps.tile([128, TQ], F32, tag="mm", bufs=2, name="cp")
                    for tap in range(9):
                        ky, kx = tap // 3, tap % 3
                        rr = _win(xTpad[ct], tt * 8, ky - 1, kx - 1)
                        nc.tensor.matmul(cp, dwdiag[:, tap * 2 + ct, :],
                                         rr, start=(tap == 0), stop=(tap == 8))
                    y_ = work.tile([128, TQ], F32R, tag=f"ysb{ct}", name=f"ysb{ct}")
                    nc.scalar.activation(out=y_, in_=cp, func=AF.Identity, bias=dwB[ct], scale=1.0)
                    y2_ = work.tile([128, TQ], F32R, tag=f"y2sb{ct}", name=f"y2sb{ct}")
                    nc.scalar.activation(out=y2_, in_=cp, func=AF.Square, bias=dwB[ct], scale=1.0)
                    ysb.append(y_); y2sb.append(y2_)
                sp = psS.tile([128, TQ], F32, tag="s", bufs=1, name="sp")
                s2p = psS.tile([128, TQ], F32, tag="s2", bufs=1, name="s2p")
                for ct in range(2):
                    nc.tensor.matmul(sp, ones, ysb[ct], start=(ct == 0), stop=(ct == 1))
                    nc.tensor.matmul(s2p, ones, y2sb[ct], start=(ct == 0), stop=(ct == 1))
                mn = work.tile([128, TQ], F32, tag="lnm", name="lnm")
                nc.vector.tensor_scalar(out=mn, in0=sp, scalar1=1.0 / C, scalar2=None, op0=OP.mult)
                msq = work.tile([128, TQ], F32, tag="lnmsq", name="lnmsq")
                nc.vector.tensor_mul(out=msq, in0=mn, in1=mn)
                var = work.tile([128, TQ], F32, tag="lnvar", name="lnvar")
                nc.vector.scalar_tensor_tensor(out=var, in0=s2p, scalar=1.0 / C, in1=msq,
                                               op0=OP.mult, op1=OP.subtract)
                sd = work.tile([128, TQ], F32, tag="lnsd", name="lnsd")
                nc.scalar.activation(out=sd, in_=var, func=AF.Sqrt, bias=epsT, scale=1.0)
                rstd = work.tile([128, TQ], F32, tag="lnrstd", name="lnrstd")
                nc.vector.reciprocal_approx_fast(out=rstd, in_=sd)
                for ct in range(2):
                    t1 = work.tile([128, TQ], F32, tag="lnmsq", name="t1")
                    nc.vector.tensor_sub(out=t1, in0=ysb[ct].bitcast(F32), in1=mn)
                    t2 = work.tile([128, TQ], F32, tag="lnvar", name="t2")
                    nc.vector.tensor_mul(out=t2, in0=t1, in1=rstd)
                    nc.scalar.activation(out=x1[ct][:, tt * TQ:(tt + 1) * TQ], in_=t2,
                                         func=AF.Gelu, bias=lnB[ct], scale=lnG[ct])

        xstack.close()

        # ---- per-chunk pipeline ----
        qp = ctx.enter_context(tc.tile_pool(name="qp", bufs=1))
        sm = ctx.enter_context(tc.tile_pool(name="sm", bufs=2))
        psOut = ctx.enter_context(tc.tile_pool(name="psOut", bufs=1, space="PSUM"))
        psW = ctx.enter_context(tc.tile_pool(name="psW", bufs=1, space="PSUM"))

        def front(ch):
            """heads -> softmax -> tents -> packed q products for chunk ch."""
            x1sl = [x1[kt][:, ch * TQ:(ch + 1) * TQ] for kt in range(2)]
            # oxy[pi]: [_,2,512] offsets (x,y); em: exp(mask logits)
            oxy = [qp.tile([128, 2, TQ], BF16, tag="oxy0", bufs=1, name="oxy0"),
                   qp.tile([16, 2, TQ], BF16, tag="oxy1", bufs=1, name="oxy1")]
            em = [qp.tile([128, TQ], BF16, tag="em0", bufs=2, name="em0"),
                  qp.tile([16, TQ], BF16, tag="em1", bufs=2, name="em1")]
            for sec in range(3):
                for pi, m0, msz in PR:
                    hp = ps.tile([128, TQ], F32, tag="mm", bufs=2, name="hp")
                    for kt in range(2):
                        nc.tensor.matmul(hp[:msz], headWT[:, kt, sec * 160 + m0: sec * 160 + m0 + msz],
                                         x1sl[kt], start=(kt == 0), stop=(kt == 1))
                    if sec < 2:
                        nc.scalar.activation(out=oxy[pi][:msz, sec, :], in_=hp[:msz],
                                             func=AF.Identity, bias=headB[sec][pi], scale=1.0)
                    else:
                        nc.scalar.activation(out=em[pi][:msz], in_=hp[:msz],
                                             func=AF.Exp, bias=headB[sec][pi], scale=1.0)
            # softmax denominators per group, replicated to (g,p) rows
            den = [ps.tile([128, TQ], F32, tag="mm", bufs=2, name="dn0"),
                   ps.tile([16, TQ], F32, tag="mm", bufs=2, name="dn1")]
            for pii, (pi, m0, msz) in enumerate(PR):
                for kt, (kpi, k0, ksz) in enumerate(PR):
                    nc.tensor.matmul(den[pii][:msz], blk[kpi][:, m0:m0 + msz], em[kpi],
                                     start=(kt == 0), stop=(kt == 1))
            rs = [qp.tile([128, TQ], F32, tag="rs0", bufs=1, name="rs0"),
                  qp.tile([16, TQ], F32, tag="rs1", bufs=1, name="rs1")]
            for pi, m0, msz in PR:
                nc.vector.reciprocal_approx_fast(out=rs[pi][:msz], in_=den[pi][:msz])
            mt_ = [qp.tile([128, TQ], BF16, tag="mt0", bufs=1, name="mt0"),
                   qp.tile([16, TQ], BF16, tag="mt1", bufs=1, name="mt1")]
            for pi, m0, msz in PR:
                nc.vector.tensor_mul(out=mt_[pi][:msz], in0=em[pi][:msz], in1=rs[pi][:msz])
            # tents T[pi]: [_, cand(3), axis(2), t]; cand1 center negated (|o|-1)
            TT = [qp.tile([128, 3, 2, TQ], BF16, tag="T0", bufs=1, name="T0"),
                  qp.tile([16, 3, 2, TQ], BF16, tag="T1", bufs=1, name="T1")]
            for pi, m0, msz in PR:
                t = TT[pi]
                nc.vector.tensor_scalar(out=t[:msz, 0, :, :], in0=oxy[pi][:msz],
                                        scalar1=-1.0, scalar2=0.0, op0=OP.mult, op1=OP.max)
                nc.vector.tensor_scalar(out=t[:msz, 2, :, :], in0=oxy[pi][:msz],
                                        scalar1=0.0, scalar2=None, op0=OP.max)
                nc.vector.scalar_tensor_tensor(out=t[:msz, 1, :, :], in0=t[:msz, 0, :, :],
                                               scalar=1.0, in1=t[:msz, 2, :, :],
                                               op0=OP.subtract, op1=OP.add)
            # mty3[pi]: [_, cy(3), t] = mt * ty_cy  (stride-0 expand of mt)
            mty3 = [qp.tile([128, 3, TQ], BF16, tag="mty0", bufs=1, name="mty0"),
                    qp.tile([16, 3, TQ], BF16, tag="mty1", bufs=1, name="mty1")]
            for pi, m0, msz in PR:
                mtx = mt_[pi][:msz].unsqueeze(1)
                aa = copy.copy(mtx.ap)
                aa[1] = [0, 3]
                mtx3 = copy.copy(mtx)
                mtx3.ap = aa
                tyv = TT[pi][:msz, :, 1, :]
                nc.vector.tensor_mul(out=mty3[pi][:msz], in0=mtx3, in1=tyv)
            # q products: qm3[cy] [128, cx(3), t]; leftovers dense in qL/qL2
            qm3 = [qp.tile([128, 3, TQ], BF16, tag=f"qm{cy}", bufs=2, name=f"qm{cy}")
                   for cy in range(3)]
            txv0 = TT[0][:, :, 0, :]
            for cy in range(3):
                mv = mty3[0][:, cy, :].unsqueeze(1)
                aa = copy.copy(mv.ap)
                aa[1] = [0, 3]
                mv3 = copy.copy(mv)
                mv3.ap = aa
                nc.vector.tensor_mul(out=qm3[cy], in0=mv3, in1=txv0)
            qL = [qp.tile([128, TQ], BF16, tag="qLa", bufs=2, name="qLa"),
                  qp.tile([128, TQ], BF16, tag="qLb", bufs=2, name="qLb"),
                  qp.tile([16, TQ], BF16, tag="qLc", bufs=2, name="qLc")]
            for cand in range(9):
                cy, cx = cand // 3, cand % 3
                src0 = mty3[1][:, cy, :]
                src1 = TT[1][:, cx, 0, :]
                if cand < 8:
                    o = (cand % 4) * 32
                    nc.vector.tensor_mul(out=qL[cand // 4][o:o + 16], in0=src0, in1=src1)
                else:
                    nc.vector.tensor_mul(out=qL[2], in0=src0, in1=src1)
            return qm3, qL

        def backA(ch, qm3, qL):
            """scand matmuls -> wsb -> broadcast DMAs into wball."""
            wsb = qp.tile([128, 4, TQ], BF16, tag="wsb", bufs=2, name="wsb")
            for j in range(4):
                msz = 128 if j < 3 else 16
                wp = ps.tile([128, TQ], F32, tag="mm", bufs=2, name="wp")
                for k in range(12):
                    if k < 9:
                        rhs = qm3[k // 3][:, k % 3, :]
                        lhs = scandW[:, k, j * 128:j * 128 + msz]
                    elif k < 11:
                        rhs = qL[k - 9]
                        lhs = scandW[:, k, j * 128:j * 128 + msz]
                    else:
                        rhs = qL[2]
                        lhs = scandW[0:16, 11, j * 128:j * 128 + msz]
                    nc.tensor.matmul(wp[:msz], lhs, rhs, start=(k == 0), stop=(k == 11))
                nc.scalar.copy(out=wsb[:msz, j, :], in_=wp[:msz])
            # one shared tag, bufs=3: cycles over (chunk, ct) pairs
            wball = [qp.tile([128, 25 * TQ], BF16, tag="wball", bufs=2, name=f"wball{ct}")
                     for ct in range(2)]
            # all 25 slots via PE selection matmuls, drained to SBUF in pairs
            for ct in range(2):
                i = 0
                while i < 25:
                    npl = min(2, 25 - i)
                    wbt = psW.tile([128, 2, TQ], F32, tag="wb", bufs=2, name="wb")
                    for jj in range(npl):
                        s = i + jj
                        if s < 24:
                            q, j = s // 3, s % 3
                            gi, ksz = q, 128
                        else:
                            j, gi, ksz = 3, 8, 16
                        nc.tensor.matmul(wbt[:, jj, :],
                                         gsel[0:ksz, gi, ct * 128:(ct + 1) * 128],
                                         wsb[0:ksz, j, :], start=True, stop=True)
                    nc.scalar.copy(out=_r(wball[ct][:, i * TQ:(i + npl) * TQ],
                                          "p (s t) -> p s t", t=TQ),
                                   in_=wbt[:, 0:npl, :])
                    i += npl
            return wball

        def backB(ch, wball):
            """apply windows, partial tree, out_proj with absorbed planes."""
            for ct in range(2):
                wb = wball[ct]
                for iu in range(5):
                    u = iu - 2
                    wv = _strided(wb, (3 * iu) * TQ, [(TQ, 3), (64, 8), (1, WD)])
                    xv = _win3(xppod[ct], ch * 8, u, -2, 3, odd=True)
                    nc.vector.tensor_mul(out=wv, in0=wv, in1=xv)
                    wv2 = _strided(wb, (15 + 2 * iu) * TQ, [(TQ, 2), (64, 8), (1, WD)])
                    xv2 = _win3(xppad[ct], ch * 8, u, -1, 2, odd=False)
                    nc.vector.tensor_mul(out=wv2, in0=wv2, in1=xv2)
                # tree over slots [16,25): 9 slots -> acc at slot 16
                def wsl(a, b):
                    return wb[:, a * TQ:b * TQ]
                nc.vector.tensor_add(out=wsl(13, 19), in0=wsl(13, 19), in1=wsl(19, 25))
                nc.vector.tensor_add(out=wsl(13, 16), in0=wsl(13, 16), in1=wsl(16, 19))
                nc.vector.tensor_add(out=wsl(13, 14), in0=wsl(13, 14), in1=wsl(14, 15))
                nc.vector.tensor_add(out=wsl(13, 14), in0=wsl(13, 14), in1=wsl(15, 16))
            # out_proj: absorb slots 0..TREE_START-1, tree accs (slot 16) LAST
            if os.environ.get("V2_NOABSORB"):
                seq = [(0, 16), (1, 16)]
            else:
                seq = ([(ct, s) for ct in range(2) for s in range(TREE_START)]
                       + [(0, 13), (1, 13)])
            for mt in range(2):
                op_ = psOut.tile([128, TQ], F32, tag="op", bufs=2, name="op")
                nsteps = len(seq)
                step = 0
                for ct, s in seq:
                    lhs = outWT[:, ct, mt * 128:(mt + 1) * 128]
                    rhs = wball[ct][:, s * TQ:(s + 1) * TQ]
                    nc.tensor.matmul(op_, lhs, rhs,
                                     start=(step == 0), stop=(step == nsteps - 1))
                    step += 1
                o_ = sm.tile([128, TQ], F32, tag=f"osb{mt}", bufs=1, name=f"osb{mt}")
                nc.scalar.activation(out=o_, in_=op_, func=AF.Identity, bias=outB[mt], scale=1.0)
                nc.sync.dma_start(out=out_d[mt * 128:(mt + 1) * 128, ch * TQ:(ch + 1) * TQ], in_=o_)

        for _ in range(2):
            nc.vector.memset(qp.tile([128, TQ], BF16, tag="qLa", bufs=2, name="qLa_i"), 0.0)
            nc.vector.memset(qp.tile([128, TQ], BF16, tag="qLb", bufs=2, name="qLb_i"), 0.0)

        pend = None
        for ch in range(NCH):
            if pend is not None:
                backB(ch - 1, pend)
            q = front(ch)
            pend = backA(ch, *q)
        backB(NCH - 1, pend)

    return nc


# ---------------- host side ----------------
_BUILT = {}


def _get_built(debug=False):
    key = bool(debug)
    if key not in _BUILT:
        nc = build(debug=debug)
        nc.compile()
        _BUILT[key] = nc
    return _BUILT[key]


def prep_weights(inputs):
    f32 = np.float32
    dw_w = np.asarray(inputs["dw_w"], f32)
    off_w = np.asarray(inputs["off_w"], f32)
    mask_w = np.asarray(inputs["mask_w"], f32)
    in_w = np.asarray(inputs["in_w"], f32)
    out_w = np.asarray(inputs["out_w"], f32)

    dwdiag = np.zeros((128, 18, 128), f32)
    cl = np.arange(128)
    for tap in range(9):
        ky, kx = tap // 3, tap % 3
        for ct in range(2):
            dwdiag[cl, tap * 2 + ct, cl] = dw_w[ct * 128:(ct + 1) * 128, 0, ky, kx]

    headWT = np.zeros((C, 480), f32)
    headB = np.zeros((480,), f32)
    off_b = np.asarray(inputs["off_b"], f32)
    mask_b = np.asarray(inputs["mask_b"], f32)
    for g in range(G):
        for p in range(P):
            r = g * P + p
            headWT[:, 0 * 160 + r] = off_w[g * 18 + p * 2 + 0]
            headWT[:, 1 * 160 + r] = off_w[g * 18 + p * 2 + 1]
            headWT[:, 2 * 160 + r] = mask_w[g * 9 + p]
            headB[0 * 160 + r] = off_b[g * 18 + p * 2 + 0]
            headB[1 * 160 + r] = off_b[g * 18 + p * 2 + 1]
            headB[2 * 160 + r] = mask_b[g * 9 + p]

    # scand: 11 contraction tiles x 512 outputs (4 sections x 128 rows)
    # output column for (slot s, group g): j*128 + q*16 + g  with
    #   s<24: q=s//3, j=s%3;  s==24: q=0, j=3
    scand = np.zeros((128, 12, 400), f32)
    for p in range(P):
        for cy in range(3):
            for cx in range(3):
                cand = cy * 3 + cx
                sgn = (-1.0 if cy == 1 else 1.0) * (-1.0 if cx == 1 else 1.0)
                u_idx = DY[p] + (cy - 1) + 2
                v = DX[p] + (cx - 1)
                s = slot_of(u_idx, v)
                q, j = (s // 3, s % 3) if s < 24 else (0, 3)
                for g in range(G):
                    gp = g * 9 + p
                    col = j * 128 + q * 16 + g if j < 3 else 384 + g
                    if gp < 128:
                        scand[gp, cand, col] = sgn
                    elif cand < 8:
                        scand[(cand % 4) * 32 + (gp - 128), 9 + cand // 4, col] = sgn
                    else:
                        scand[gp - 128, 11, col] = sgn

    blk = np.zeros((144, 144), f32)
    for g in range(G):
        blk[g * P:(g + 1) * P, g * P:(g + 1) * P] = 1.0

    bvec = np.zeros((128, 16), f32)
    def put2(col, v):
        bvec[:, col] = v[0:128]
        bvec[:, col + 1] = v[128:256]
    put2(0, np.asarray(inputs["dw_b"], f32))
    put2(2, np.asarray(inputs["ln_g"], f32))
    put2(4, np.asarray(inputs["ln_b"], f32))
    put2(6, np.asarray(inputs["in_b"], f32))
    put2(8, np.asarray(inputs["out_b"], f32))
    for s in range(3):
        bvec[:, 10 + s] = headB[s * 160: s * 160 + 128]
        bvec[0:16, 13 + s] = headB[s * 160 + 128: s * 160 + 144]

    gsel = np.zeros((128, 9, C), f32)
    for gi in range(9):
        q = gi if gi < 8 else 0
        for c in range(C):
            ct, cl = c // 128, c % 128
            gsel[q * 16 + ct * 8 + cl // 16, gi, c] = 1.0

    import ml_dtypes
    tobf = lambda a: np.ascontiguousarray(a).astype(ml_dtypes.bfloat16)

    return {
        "dwdiag": tobf(dwdiag.reshape(128, 18 * 128)),
        "inWT": tobf(in_w.T),
        "headWT": tobf(headWT),
        "scand": tobf(scand.reshape(128, 12 * 400)),
        "blk": tobf(blk),
        "ones": np.ones((128, 128), f32),
        "outWT": tobf(out_w.T),
        "gsel": tobf(gsel.reshape(128, 9 * C)),
        "bvec": bvec,
    }


def kernel(**inputs):
    import ml_dtypes
    nc = _get_built(debug=False)
    wts = prep_weights(inputs)
    x = np.asarray(inputs["x"], np.float32)
    in_maps = []
    for n in range(N_CORES):
        m = dict(wts)
        m["xT"] = np.ascontiguousarray(x[n].reshape(T, C).T).astype(ml_dtypes.bfloat16)
        in_maps.append(m)
    res = bass_utils.run_bass_kernel_spmd(nc, in_maps, core_ids=list(range(N_CORES)))
    out = np.stack([np.ascontiguousarray(res.results[n]["out"].reshape(C, T).T).reshape(H, WD, C)
                    for n in range(N_CORES)])
    return out
